# revision 35
# baseline (speedup 1.0000x reference)
"""DDiT block (AdaLN-modulated transformer block) on 8 Trainium2 NeuronCores.

Sharding: pure data-parallel, core = (batch b in {0,1}) x (query-chunk k in
0..3 of 512 tokens).  Each core computes LN1/K/V over the full 2048-token
batch (K/V replicated within the 4 cores of a batch -- avoids collectives),
then attention / out-proj / LN2 / MLP for its own 512 queries.  AdaLN
modulation vectors are computed host-side and replicated.

Device layout: activations kept transposed ([d on partitions, t on free]):
  - LN stats (reduce over d) are ones-vector matmuls on the PE (with the
    1/D normalization folded into the stationary ones column),
  - rstd = exp(-0.5*ln(var+eps)) on ACT: ln+exp live in ONE activation
    table set (natural_log_exp_and_others), so the whole kernel pays only
    two ACT table loads (the second for gelu),
  - per-token rows (rstd, -mu*rstd) are broadcast across partitions with a
    K=1 outer-product matmul into PSUM (no DRAM bounce on the LN path),
  - the softmax denominator comes free from a ones-column appended to V
    (AV matmul row 64 = sum of exp).

Pipelining: the kernel is organized so the ~110us of softmax Exp on the
scalar engine overlaps matmul work instead of serializing with it:
  - phase A: LN1 software-pipelined with K/Q projection (chunk ch's
    modulate runs on DVE/ACT while chunk ch-1's K matmuls run on the PE),
  - pair-major attention: each pair's scores+exp dovetail with the same
    pair's AV at a 2-slot lag, plus just-in-time V projection; score
    matmuls for the even/odd head of a pair are emitted back-to-back with
    tile_position (0,0)/(64,0) so the two K=64 matmuls run concurrently
    in the PE array; AV output + denominator are copied out of PSUM
    immediately so the po bank ring never stalls the next pair,
  - MLP1 (weights streamed in chunks) then MLP2 wt-major, so each output
    chunk's epilogue + store DMA starts as soon as it is accumulated.

prec="bf16" runs the big GEMMs in bf16; LN statistics, softmax and residual
accumulation stay fp32/fp32r.
"""

import contextlib

import numpy as np

import concourse.bass as bass
import concourse.mybir as mybir
import concourse.tile as tile
from concourse.bass_utils import run_bass_kernel_spmd

F32 = mybir.dt.float32
F32R = mybir.dt.float32r
BF16 = mybir.dt.bfloat16
AF = mybir.ActivationFunctionType
OP = mybir.AluOpType

D = 768
S = 2048
H = 12
DH = 64
DC = D // 128           # 6 chunks of d on partitions
HID = 4 * D             # 3072
HC = HID // 128         # 24
NQ = 512                # queries per core
NCH = S // NQ           # 4 token chunks
NTP = S // 128          # 16 key chunks of 128
NP = H // 2             # 6 head pairs
EPS = 1e-5


def _bcast_row(nc, dbc, dst, row):
    """Broadcast a [1, N] SBUF row across partitions of dst via a DRAM bounce
    (this walrus build cannot encode the gpsimd PartitionBroadcast ISA)."""
    scratch = dbc.tile([1, row.shape[-1]], F32, tag="bc", bufs=4)
    nc.sync.dma_start(scratch, row)
    src = bass.AP(tensor=scratch.tensor, offset=scratch.offset,
                  ap=[[0, dst.shape[0]]] + list(scratch.ap[1:]))
    nc.sync.dma_start(dst, src)


def _body(tc, dram, gelu_mode, prec):
    nc = tc.nc
    assert prec == "bf16"
    MDT = BF16
    r128 = lambda name: dram[name].ap().rearrange("(o p) j -> p o j", p=128)
    xT_r = r128("xT")
    xTb_r = r128("xTb")
    wqkv_r = r128("w_qkvT")
    wout_r = r128("w_outT")
    wm1_r = r128("w_m1T")
    wm2_r = r128("w_m2T")
    outT_r = r128("outT")

    with contextlib.ExitStack() as ctx:
        main = ctx.enter_context(tc.tile_pool(name="main", bufs=1))
        psmm = ctx.enter_context(tc.tile_pool(name="psmm", bufs=2, space="PSUM"))
        rows = ctx.enter_context(tc.tile_pool(name="rows", bufs=1))
        vt = ctx.enter_context(tc.tile_pool(name="vt", bufs=1))

        # ---- first x chunk + K weights first (per-o so LN stats start
        # as soon as the first 128-partition slice lands); the small
        # constant DMAs would otherwise serialize ahead of them on SP
        x0 = main.tile([128, DC, NQ], MDT, name="x0", tag="xr", bufs=2)
        for o in range(DC):
            nc.sync.dma_start(x0[:, o, :], xTb_r[:, o, 0:NQ])

        # ---- constants / small tensors
        sums_f = main.tile([128, 1], F32)
        nc.vector.memset(sums_f, 1.0 / D)
        sums_col = main.tile([128, 1], MDT)
        nc.vector.tensor_copy(sums_col, sums_f)
        sums_colr = main.tile([128, 1], F32R)
        nc.vector.tensor_copy(sums_colr, sums_f)
        ones_f32 = main.tile([128, 1], F32)
        nc.vector.memset(ones_f32, 1.0)
        bc_row = main.tile([1, 128], MDT)
        nc.vector.tensor_copy(bc_row, ones_f32[0:1, :].to_broadcast((1, 128)))
        ada = main.tile([128, 36], F32)
        nc.sync.dma_start(ada, dram["ada_c"].ap())
        n1_sb = main.tile([128, DC], F32)
        nc.sync.dma_start(n1_sb, dram["n1_c"].ap())
        n2_sb = main.tile([128, DC], F32)
        nc.sync.dma_start(n2_sb, dram["n2_c"].ap())
        b1_sb = main.tile([128, HC], F32)
        nc.sync.dma_start(b1_sb, dram["b1_c"].ap())
        b2_sb = main.tile([128, DC], F32)
        nc.sync.dma_start(b2_sb, dram["b2_c"].ap())

        sh_msa, sc_msa, g_msa = ada[:, 0:6], ada[:, 6:12], ada[:, 12:18]
        sh_mlp, sc_mlp, g_mlp = ada[:, 18:24], ada[:, 24:30], ada[:, 30:36]
        a1 = main.tile([128, DC], F32)
        nc.vector.scalar_tensor_tensor(a1, in0=sc_msa, scalar=1.0, in1=n1_sb,
                                       op0=OP.add, op1=OP.mult)
        a2 = main.tile([128, DC], F32)
        nc.vector.scalar_tensor_tensor(a2, in0=sc_mlp, scalar=1.0, in1=n2_sb,
                                       op0=OP.add, op1=OP.mult)
        gb2 = main.tile([128, DC], F32)
        nc.vector.tensor_mul(gb2, g_mlp, b2_sb)

        oT = main.tile([128, DC, NQ], MDT, name="oT")
        x2 = main.tile([128, DC, NQ], F32R, name="x2")
        xskip = main.tile([128, DC, NQ], F32R, name="xskip")
        xm2 = main.tile([128, DC, NQ], MDT, name="xm2")
        wout_all = main.tile([128, DC, D], MDT, name="wout")

        attn_ctx = tc.tile_pool(name="attn", bufs=1)
        attn = attn_ctx.__enter__()
        kt = [attn.tile([128, S], MDT, name=f"kt{p}") for p in range(NP)]
        qT = attn.tile([128, NP, NQ], MDT, name="qT")
        v_tiles = [attn.tile([128, H, DH + 1], MDT, name=f"v{tp}")
                   for tp in range(NTP)]
        xm_tiles = []

        wvp_ctx = tc.tile_pool(name="wvp", bufs=1)
        wvp = wvp_ctx.__enter__()
        wv_all = wvp.tile([128, DC, D], MDT, name="wv")
        wqk_ctx = tc.tile_pool(name="wqk", bufs=1)
        wqk = wqk_ctx.__enter__()
        wq_all = wqk.tile([128, DC, D], MDT, name="wq")
        wk_all = wqk.tile([128, DC, D], MDT, name="wk")

        def _ln_stats(psr, x_t, s1_lhs=None, sq_eng=None):
            """LN sums via PE matmuls (1/D folded into the stationary ones
            column); squares on gpsimd (idle engine) unless sq_eng given."""
            eng = sq_eng if sq_eng is not None else nc.vector
            s1 = psr.tile([1, NQ], F32, tag="s1")
            for o in range(DC):
                nc.tensor.matmul(s1, s1_lhs if s1_lhs is not None else sums_col,
                                 x_t[:, o, :],
                                 start=(o == 0), stop=(o == DC - 1))
            s2 = psr.tile([1, NQ], F32, tag="s2")
            for o in range(DC):
                sq = vt.tile([128, NQ], MDT, tag="sq", bufs=2)
                eng.tensor_mul(sq, x_t[:, o, :], x_t[:, o, :])
                nc.tensor.matmul(s2, sums_col, sq,
                                 start=(o == 0), stop=(o == DC - 1))
            return s1, s2

        def _ln_finish(psr, s1, s2, x_t, xm, a_col, sh_col):
            """rstd = exp(-0.5*ln(var+eps)) on ACT (one table set), rows
            broadcast across partitions with K=1 matmuls into PSUM, then
            xm[:, o, :] = ((x - mu) * rstd) * a[d] + sh[d] (DVE + ACT)."""
            mu = rows.tile([1, NQ], F32R, tag="mu")
            nc.vector.tensor_copy(mu, s1)
            musq = rows.tile([1, NQ], F32R, tag="musq")
            nc.vector.tensor_mul(musq, mu, mu)
            var = rows.tile([1, NQ], F32R, tag="var")
            nc.vector.scalar_tensor_tensor(var, in0=s2, scalar=EPS, in1=musq,
                                           op0=OP.add, op1=OP.subtract)
            lnv = rows.tile([1, NQ], F32R, tag="lnv")
            nc.scalar.activation(lnv, var, AF.Ln)
            rstd = rows.tile([1, NQ], MDT, tag="rstd")
            nc.scalar.activation(rstd, lnv, AF.Exp, bias=0.0, scale=-0.5)
            negmr = rows.tile([1, NQ], MDT, tag="negmr")
            nc.vector.scalar_tensor_tensor(negmr, in0=mu, scalar=-1.0,
                                           in1=rstd, op0=OP.mult, op1=OP.mult)
            Rb = psr.tile([128, NQ], F32, tag="Rb")
            nc.tensor.matmul(Rb, bc_row, rstd, start=True, stop=True)
            Mb = psr.tile([128, NQ], F32, tag="Mb")
            nc.tensor.matmul(Mb, bc_row, negmr, start=True, stop=True)
            for o in range(DC):
                t = vt.tile([128, NQ], F32R, tag="tu", bufs=4)
                nc.vector.tensor_mul(t, x_t[:, o, :], Rb)
                u = vt.tile([128, NQ], F32R, tag="tu", bufs=4)
                nc.vector.tensor_add(u, t, Mb)
                nc.scalar.activation(xm[:, o, :], u, AF.Identity,
                                     bias=sh_col[:, o:o + 1],
                                     scale=a_col[:, o:o + 1])

        def _scores_exp(p, kc, sc_pool):
            """Paired score matmuls (even head rows 0-63, odd head rows
            64-127 -> adjacent MMs in different PSUM banks, concurrent in
            the PE array via row tiling) + one [128, 1024] exp."""
            sc = sc_pool.tile([128, 2 * NQ], F32, tag="sc", bufs=2)
            nc.tensor.matmul(sc[:, 0:NQ],
                             kt[p][0:DH, kc * 128:(kc + 1) * 128],
                             qT[0:DH, p, :], start=True, stop=True)
            nc.tensor.matmul(sc[:, NQ:2 * NQ],
                             kt[p][DH:128, kc * 128:(kc + 1) * 128],
                             qT[DH:128, p, :], start=True, stop=True)
            et = attn.tile([128, 2 * NQ], MDT, tag="et", bufs=4)
            nc.scalar.activation(et, sc, AF.Exp, bias=0.0, scale=0.125)
            return et

        def _k_proj(p, ch, xm):
            ps = psmm.tile([128, NQ], F32, tag="mm")
            for o in range(DC):
                nc.tensor.matmul(ps, wk_all[:, o, p * 128:(p + 1) * 128],
                                 xm[:, o, :], start=(o == 0), stop=(o == DC - 1))
            nc.scalar.copy(kt[p][:, ch * NQ:(ch + 1) * NQ], ps)

        def _q_proj(p, xm0):
            ps = psmm.tile([128, NQ], F32, tag="mm")
            for o in range(DC):
                nc.tensor.matmul(ps, wq_all[:, o, p * 128:(p + 1) * 128],
                                 xm0[:, o, :], start=(o == 0), stop=(o == DC - 1))
            nc.vector.tensor_copy(qT[:, p, :], ps)

        def _v_proj(tp, half):
            """V for key-chunk tp, heads [6*half, 6*half+6), natural layout."""
            ch, sub = tp // 4, tp % 4
            ps = psmm.tile([128, 384], F32, tag="mm")
            for o in range(DC):
                nc.tensor.matmul(
                    ps, xm_tiles[ch][:, o, sub * 128:(sub + 1) * 128],
                    wv_all[:, o, half * 384:(half + 1) * 384],
                    start=(o == 0), stop=(o == DC - 1))
            nc.vector.tensor_copy(
                v_tiles[tp][:, half * 6:(half + 1) * 6, 0:DH],
                ps.rearrange("p (h d) -> p h d", h=6))

        def _av_step(p, e, kc, po, ets):
            """One AV accumulation matmul for head 2p+e, key-chunk kc."""
            h = 2 * p + e
            nc.tensor.matmul(po, v_tiles[kc][:, h, :],
                             ets[kc][:, e * NQ:(e + 1) * NQ],
                             start=(kc == 0), stop=(kc == NTP - 1))

        def _po_evac(po):
            """Copy AV output + denominator row out of PSUM immediately so
            the po bank frees for the next pair."""
            poS = vt.tile([DH, NQ], MDT, tag="poS", bufs=2)
            nc.vector.tensor_copy(poS, po[0:DH, :])
            denS = vt.tile([1, NQ], F32, tag="denS", bufs=2)
            nc.vector.tensor_copy(denS, po[DH:DH + 1, :])
            return poS, denS

        def _head_tail(p, e, poS, denS):
            """Normalize: oT[head rows, p, :] = poS / den.  1/den on ACT as
            exp(-ln(den)) (same table set as softmax exp) for pairs where
            ACT has slack, DVE reciprocal mid-window; the row is broadcast
            across partitions with a K=1 matmul into PSUM (the DRAM-bounce
            broadcast costs ~7us of SP issue time and serialized the
            attention->out-projection transition)."""
            rrow_bf = vt.tile([1, NQ], MDT, tag="rrowb", bufs=2)
            lnd = vt.tile([1, NQ], F32R, tag="lnd", bufs=1)
            nc.scalar.activation(lnd, denS, AF.Ln)
            nc.scalar.activation(rrow_bf, lnd, AF.Exp, bias=0.0,
                                 scale=-1.0)
            rb_ps = psmm.tile([DH, NQ], F32, tag="mm")
            nc.tensor.matmul(rb_ps, bc_row[:, 0:DH], rrow_bf,
                             start=True, stop=True)
            nc.vector.tensor_mul(oT[e * DH:(e + 1) * DH, p, :], poS, rb_ps)

        # ones column for the softmax denominator
        for tp in range(NTP):
            nc.vector.tensor_copy(v_tiles[tp][:, :, DH:DH + 1],
                                  ones_f32.to_broadcast((128, H, 1)))

        # =========== phase A: LN1 software-pipelined with K-projection
        # (K/Q for chunk ch-1 run on the PE while chunk ch's modulate runs
        # on DVE/ACT, so the PE never waits for the LN chain)
        q_sched = {0: [], 1: [0, 1], 2: [2, 4], 3: [3, 5]}
        with tc.tile_pool(name="psA", bufs=1, space="PSUM") as psA:
            for ch in range(NCH):
                if ch == 0:
                    x_t = x0
                else:
                    x_t = main.tile([128, DC, NQ], MDT, name=f"x{ch}",
                                    tag="xr", bufs=2)
                    nc.sync.dma_start(x_t, xTb_r[:, :, ch * NQ:(ch + 1) * NQ])
                if ch == 0:
                    nc.sync.dma_start(wk_all[:, :, 0:384],
                                        wqkv_r[:, :, D:D + 384])
                    nc.sync.dma_start(wk_all[:, :, 384:768],
                                        wqkv_r[:, :, D + 384:2 * D])
                elif ch == 1:
                    nc.sync.dma_start(wq_all, wqkv_r[:, :, 0:D])
                elif ch == 2:
                    nc.sync.dma_start(wv_all, wqkv_r[:, :, 2 * D:3 * D])
                    nc.sync.dma_start(wout_all, wout_r)
                xm = attn.tile([128, DC, NQ], MDT, name=f"xm{ch}")
                s1, s2 = _ln_stats(psA, x_t)
                _ln_finish(psA, s1, s2, x_t, xm, a1, sh_msa)
                xm_tiles.append(xm)
                if ch >= 1:
                    for p in range(NP):
                        _k_proj(p, ch - 1, xm_tiles[ch - 1])
                    for p in q_sched[ch]:
                        _q_proj(p, xm_tiles[0])
                    if ch == 3:
                        for tp in range(4):
                            _v_proj(tp, 0)
            for p in range(NP):
                _k_proj(p, NCH - 1, xm_tiles[NCH - 1])
        wqk_ctx.__exit__(None, None, None)

        # =========== pair-major attention: scores+exp dovetail with the
        # same pair's AV (lag 2) and just-in-time V projection (p0: half0,
        # p1: half1)
        with tc.tile_pool(name="scP", bufs=2, space="PSUM") as scP, \
             tc.tile_pool(name="pop", bufs=2, space="PSUM") as pop:
            for p in range(NP):
                if p == 2:
                    nc.sync.dma_start(xskip, xT_r[:, :, 0:NQ])
                ets = [None] * NTP
                po_e = pop.tile([DH + 1, NQ], F32, tag="po")
                po_o = pop.tile([DH + 1, NQ], F32, tag="po")
                for s in range(NTP + 2):
                    if s < NTP:
                        ets[s] = _scores_exp(p, s, scP)
                        if p == 0 and s >= 4:
                            _v_proj(s, 0)
                        elif p == 1 and s % 2 == 0:
                            _v_proj(s, 1)
                        elif p == 2 and s % 2 == 1:
                            _v_proj(s, 1)
                    if s >= 2:
                        _av_step(p, 0, s - 2, po_e, ets)
                        _av_step(p, 1, s - 2, po_o, ets)
                poS_e, den_e = _po_evac(po_e)
                poS_o, den_o = _po_evac(po_o)
                _head_tail(p, 0, poS_e, den_e)
                _head_tail(p, 1, poS_o, den_o)
        wvp_ctx.__exit__(None, None, None)
        attn_ctx.__exit__(None, None, None)

        # =========== out-projection + gated residual -> x2 (f32r), with
        # LN2 stats accumulating as each x2 chunk lands
        with tc.tile_pool(name="psB", bufs=1, space="PSUM") as psB:
            s1B = psB.tile([1, NQ], F32, tag="s1")
            s2B = psB.tile([1, NQ], F32, tag="s2")
            for mo in range(DC):
                ps = psmm.tile([128, NQ], F32, tag="mm")
                for o in range(DC):
                    nc.tensor.matmul(ps,
                                     wout_all[:, o, mo * 128:(mo + 1) * 128],
                                     oT[:, o, :],
                                     start=(o == 0), stop=(o == DC - 1))
                nc.vector.scalar_tensor_tensor(
                    x2[:, mo, :], in0=ps, scalar=g_msa[:, mo:mo + 1],
                    in1=xskip[:, mo, :], op0=OP.mult, op1=OP.add)
                nc.tensor.matmul(s1B, sums_colr, x2[:, mo, :],
                                 start=(mo == 0), stop=(mo == DC - 1))
                sq = vt.tile([128, NQ], MDT, tag="sq", bufs=2)
                nc.vector.tensor_mul(sq, x2[:, mo, :], x2[:, mo, :])
                nc.tensor.matmul(s2B, sums_col, sq,
                                 start=(mo == 0), stop=(mo == DC - 1))

            # =========== LN2 + modulation -> xm2
            _ln_finish(psB, s1B, s2B, x2, xm2, a2, sh_mlp)

        # =========== MLP: MLP1+gelu / MLP2 interleaved per hidden chunk
        gfunc = (AF.Gelu_apprx_tanh if gelu_mode == "fused" else AF.Gelu)
        with tc.tile_pool(name="mlpw", bufs=1) as mlpw, \
             tc.tile_pool(name="pso", bufs=1, space="PSUM") as pso:
            w1c = [None] * DC

            def w1dma(wt):
                w1c[wt] = mlpw.tile([128, DC, NQ], MDT, tag="w1", bufs=3,
                                    name=f"w1c{wt}")
                nc.sync.dma_start(w1c[wt],
                                  wm1_r[:, :, wt * NQ:(wt + 1) * NQ])

            w1dma(0)
            w1dma(1)
            w2_all = mlpw.tile([128, HC, D], MDT, name="w2")
            nc.sync.dma_start(w2_all, wm2_r)
            h_tiles = [mlpw.tile([128, NQ], MDT, name=f"h{ho}")
                       for ho in range(HC)]
            for ho in range(HC):
                wt = ho // 4
                if ho % 4 == 0 and wt + 2 < DC:
                    w1dma(wt + 2)
                ps = psmm.tile([128, NQ], F32, tag="mm")
                for o in range(DC):
                    nc.tensor.matmul(
                        ps, w1c[wt][:, o, (ho % 4) * 128:(ho % 4 + 1) * 128],
                        xm2[:, o, :], start=(o == 0), stop=(o == DC - 1))
                nc.scalar.activation(h_tiles[ho], ps, gfunc,
                                     bias=b1_sb[:, ho:ho + 1], scale=1.0)
            # MLP2 wt-major: each output chunk finishes (and its epilogue +
            # store DMA starts) after 1/6 of MLP2 instead of all at the end
            for wt2 in range(DC):
                ps2 = pso.tile([128, NQ], F32, tag="po2", bufs=2)
                for ho in range(HC):
                    nc.tensor.matmul(ps2,
                                     w2_all[:, ho, wt2 * 128:(wt2 + 1) * 128],
                                     h_tiles[ho],
                                     start=(ho == 0), stop=(ho == HC - 1))
                tmp = vt.tile([128, NQ], F32R, tag="tu", bufs=4)
                nc.scalar.activation(tmp, ps2, AF.Identity,
                                     bias=gb2[:, wt2:wt2 + 1],
                                     scale=g_mlp[:, wt2:wt2 + 1])
                nc.vector.tensor_add(x2[:, wt2, :], tmp, x2[:, wt2, :])
                nc.sync.dma_start(outT_r[:, wt2, :], x2[:, wt2, :])


def _fix_module_for_walrus(nc):
    """Workarounds for this container's walrus build:
    (a) it rejects >1 sync-wait per instruction ("Too many sync wait
        commands") -> hoist extra waits onto NoOp carrier instructions;
    (b) it rejects custom Pool InstISA ("ISA wrong length") -> expand the
        tail EVENT_SEMAPHORE_RANGE_CLEAR into per-sem sem-sub-imm updates
        using the final values observed in earlier waits.
    """
    import bass_rust
    nid = [0]

    def carrier(engine, wait):
        nop = mybir.InstNoOp(name=f"wsplit_{nid[0]}", ins=[], outs=[])
        nid[0] += 1
        nop.engine = engine
        nop.sync_info = mybir.SyncInfo(on_wait=[wait], on_update=[])
        return nop

    for f in nc.m.functions:
        new_blocks = []
        for bb in f.blocks:
            sem_final = {}
            out = []
            for inst in bb.instructions:
                si = inst.sync_info
                if si is not None:
                    for w in si.on_wait:
                        if w.sync_type == "semaphore" and w.wait_mode == "sem-ge-imm":
                            sem_final[w.id] = max(sem_final.get(w.id, 0),
                                                  w.wait_value)
                if (type(inst).__name__ == "InstISA"
                        and getattr(inst, "op_name", "") ==
                        "EVENT_SEMAPHORE_RANGE_CLEAR"):
                    ad = inst.ant_dict
                    lo, hi = ad["range_first"], ad["range_last"]
                    waits = list(si.on_wait) if si else []
                    for w in waits:
                        out.append(carrier(inst.engine, w))
                    for sem_id in range(lo, hi + 1):
                        v = sem_final.get(sem_id, 0)
                        if v == 0:
                            continue
                        ev = mybir.InstEventSemaphore(
                            name=f"semclr_{nid[0]}", ins=[], outs=[])
                        nid[0] += 1
                        ev.engine = inst.engine
                        ev.sync_info = mybir.SyncInfo(
                            on_wait=[],
                            on_update=[mybir.SyncUpdate(
                                sync_type="semaphore", id=sem_id,
                                ant_name=f"clr{sem_id}",
                                update_mode="sem-sub-imm", update_value=v,
                                update_reg=None)])
                        out.append(ev)
                    continue
                if type(inst).__name__ == "InstISA":
                    raise RuntimeError(
                        f"unsupported InstISA {getattr(inst, 'op_name', '?')}")
                waits = list(si.on_wait) if si else []
                if len(waits) > 1:
                    for w in waits[:-1]:
                        out.append(carrier(inst.engine, w))
                    inst.sync_info = mybir.SyncInfo(
                        on_wait=waits[-1:], on_update=list(si.on_update))
                out.append(inst)
            nbb = bass_rust.BasicBlock(name=bb.name, instructions=out)
            for attr in ("IsExit", "IsLoopEntry", "IsPredicated"):
                try:
                    setattr(nbb, attr, getattr(bb, attr))
                except Exception:
                    pass
            new_blocks.append(nbb)
        f.blocks = new_blocks
    return nc


def _build_nc(gelu_mode="fused", prec="bf16"):
    nc = bass.Bass(
        "TRN2", target_bir_lowering=False, debug=False, enable_asserts=False,
        num_devices=8,
    )
    WDT = BF16
    shapes = {
        "xT": ([D, S], F32R),
        "xTb": ([D, S], BF16),
        "ada_c": ([128, 36], F32),
        "n1_c": ([128, DC], F32),
        "n2_c": ([128, DC], F32),
        "w_qkvT": ([D, 3 * D], WDT),
        "w_outT": ([D, D], WDT),
        "w_m1T": ([D, HID], WDT),
        "b1_c": ([128, HC], F32),
        "w_m2T": ([HID, D], WDT),
        "b2_c": ([128, DC], F32),
    }
    dram = {k: nc.dram_tensor(k, shp, dt, kind="ExternalInput")
            for k, (shp, dt) in shapes.items()}
    dram["outT"] = nc.dram_tensor("outT", [D, NQ], F32R, kind="ExternalOutput")
    with tile.TileContext(nc) as tc:
        _body(tc, dram, gelu_mode, prec)
    return nc


def _ensure_fixed(nc):
    if not getattr(nc, "_walrus_fixed", False):
        _fix_module_for_walrus(nc)
        nc._walrus_fixed = True
    return nc


_NC_CACHE = {}


def _get_nc(gelu_mode="fused", prec="bf16"):
    key = (gelu_mode, prec)
    if key not in _NC_CACHE:
        _NC_CACHE[key] = _build_nc(gelu_mode, prec)
    return _NC_CACHE[key]


def _colpack(v, nch):
    """[nch*128] vector -> [128, nch] column-packed (col jo = v[jo*128+p])."""
    return np.ascontiguousarray(np.asarray(v, np.float32).reshape(nch, 128).T)


def make_in_maps(inputs, prec="bf16"):
    import ml_dtypes
    wdt = ml_dtypes.bfloat16
    x = np.asarray(inputs["x"], np.float32)
    c = np.asarray(inputs["c"], np.float32)
    w_ada = np.asarray(inputs["w_ada"], np.float32)
    b_ada = np.asarray(inputs["b_ada"], np.float32)
    # AdaLN modulation vectors: tiny (2x 4608x768) matmul, replicated per the
    # sharding hint; column-packed per batch.
    ada = c @ w_ada.T + b_ada                      # (2, 4608)
    tr = lambda w: np.ascontiguousarray(np.asarray(w, np.float32).T.astype(wdt))
    base = {
        "n1_c": _colpack(inputs["norm1_w"], DC),
        "n2_c": _colpack(inputs["norm2_w"], DC),
        "w_qkvT": tr(inputs["w_qkv"]),
        "w_outT": tr(inputs["w_out"]),
        "w_m1T": tr(inputs["w_mlp1"]),
        "b1_c": _colpack(inputs["b_mlp1"], HC),
        "w_m2T": tr(inputs["w_mlp2"]),
        "b2_c": _colpack(inputs["b_mlp2"], DC),
    }
    in_maps = []
    for core in range(8):
        b, k = core // 4, core % 4
        xb = np.roll(x[b], -NQ * k, axis=0)        # my queries first
        m = dict(base)
        m["xT"] = np.ascontiguousarray(xb.T)
        m["xTb"] = np.ascontiguousarray(xb.T.astype(wdt))
        m["ada_c"] = _colpack(ada[b], 36)
        in_maps.append(m)
    return in_maps


def assemble_output(results):
    out = np.empty((2, S, D), np.float32)
    for core in range(8):
        b, k = core // 4, core % 4
        out[b, NQ * k:NQ * (k + 1)] = results[core]["outT"].T
    return out


def kernel(**inputs):
    prec = "bf16"
    nc = _ensure_fixed(_get_nc(prec=prec))
    in_maps = make_in_maps(inputs, prec=prec)
    res = run_bass_kernel_spmd(nc, in_maps, core_ids=list(range(8)))
    return assemble_output(res.results)


if __name__ == "__main__":
    _get_nc()
    print("build ok")


# revision 36
# speedup vs baseline: 1.0019x; 1.0019x over previous
"""DDiT block (AdaLN-modulated transformer block) on 8 Trainium2 NeuronCores.

Sharding: pure data-parallel, core = (batch b in {0,1}) x (query-chunk k in
0..3 of 512 tokens).  Each core computes LN1/K/V over the full 2048-token
batch (K/V replicated within the 4 cores of a batch -- avoids collectives),
then attention / out-proj / LN2 / MLP for its own 512 queries.  AdaLN
modulation vectors are computed host-side and replicated.

Device layout: activations kept transposed ([d on partitions, t on free]):
  - LN stats (reduce over d) are ones-vector matmuls on the PE (with the
    1/D normalization folded into the stationary ones column),
  - rstd = exp(-0.5*ln(var+eps)) on ACT: ln+exp live in ONE activation
    table set (natural_log_exp_and_others), so the whole kernel pays only
    two ACT table loads (the second for gelu),
  - per-token rows (rstd, -mu*rstd) are broadcast across partitions with a
    K=1 outer-product matmul into PSUM (no DRAM bounce on the LN path),
  - the softmax denominator comes free from a ones-column appended to V
    (AV matmul row 64 = sum of exp).

Pipelining: the kernel is organized so the ~110us of softmax Exp on the
scalar engine overlaps matmul work instead of serializing with it:
  - phase A: LN1 software-pipelined with K/Q projection (chunk ch's
    modulate runs on DVE/ACT while chunk ch-1's K matmuls run on the PE),
  - pair-major attention: each pair's scores+exp dovetail with the same
    pair's AV at a 2-slot lag, plus just-in-time V projection; score
    matmuls for the even/odd head of a pair are emitted back-to-back with
    tile_position (0,0)/(64,0) so the two K=64 matmuls run concurrently
    in the PE array; AV output + denominator are copied out of PSUM
    immediately so the po bank ring never stalls the next pair,
  - MLP1 (weights streamed in chunks) then MLP2 wt-major, so each output
    chunk's epilogue + store DMA starts as soon as it is accumulated.

prec="bf16" runs the big GEMMs in bf16; LN statistics, softmax and residual
accumulation stay fp32/fp32r.
"""

import contextlib

import numpy as np

import concourse.bass as bass
import concourse.mybir as mybir
import concourse.tile as tile
from concourse.bass_utils import run_bass_kernel_spmd

F32 = mybir.dt.float32
F32R = mybir.dt.float32r
BF16 = mybir.dt.bfloat16
FP8 = mybir.dt.float8e4
AF = mybir.ActivationFunctionType
OP = mybir.AluOpType

D = 768
S = 2048
H = 12
DH = 64
DC = D // 128           # 6 chunks of d on partitions
HID = 4 * D             # 3072
HC = HID // 128         # 24
NQ = 512                # queries per core
NCH = S // NQ           # 4 token chunks
NTP = S // 128          # 16 key chunks of 128
NP = H // 2             # 6 head pairs
EPS = 1e-5


def _bcast_row(nc, dbc, dst, row):
    """Broadcast a [1, N] SBUF row across partitions of dst via a DRAM bounce
    (this walrus build cannot encode the gpsimd PartitionBroadcast ISA)."""
    scratch = dbc.tile([1, row.shape[-1]], F32, tag="bc", bufs=4)
    nc.sync.dma_start(scratch, row)
    src = bass.AP(tensor=scratch.tensor, offset=scratch.offset,
                  ap=[[0, dst.shape[0]]] + list(scratch.ap[1:]))
    nc.sync.dma_start(dst, src)


def _body(tc, dram, gelu_mode, prec):
    nc = tc.nc
    assert prec == "bf16"
    MDT = BF16
    r128 = lambda name: dram[name].ap().rearrange("(o p) j -> p o j", p=128)
    xT_r = r128("xT")
    xTb_r = r128("xTb")
    wqkv_r = r128("w_qkvT")
    wout_r = r128("w_outT")
    wm1_r = r128("w_m1T")
    wm2_r = r128("w_m2T")
    outT_r = r128("outT")

    with contextlib.ExitStack() as ctx:
        main = ctx.enter_context(tc.tile_pool(name="main", bufs=1))
        psmm = ctx.enter_context(tc.tile_pool(name="psmm", bufs=2, space="PSUM"))
        rows = ctx.enter_context(tc.tile_pool(name="rows", bufs=1))
        vt = ctx.enter_context(tc.tile_pool(name="vt", bufs=1))

        # ---- first x chunk + K weights first (per-o so LN stats start
        # as soon as the first 128-partition slice lands); the small
        # constant DMAs would otherwise serialize ahead of them on SP
        x0 = main.tile([128, DC, NQ], MDT, name="x0", tag="xr", bufs=2)
        for o in range(DC):
            nc.sync.dma_start(x0[:, o, :], xTb_r[:, o, 0:NQ])

        # ---- constants / small tensors
        sums_f = main.tile([128, 1], F32)
        nc.vector.memset(sums_f, 1.0 / D)
        sums_col = main.tile([128, 1], MDT)
        nc.vector.tensor_copy(sums_col, sums_f)
        sums_colr = main.tile([128, 1], F32R)
        nc.vector.tensor_copy(sums_colr, sums_f)
        ones_f32 = main.tile([128, 1], F32)
        nc.vector.memset(ones_f32, 1.0)
        bc_row = main.tile([1, 128], MDT)
        nc.vector.tensor_copy(bc_row, ones_f32[0:1, :].to_broadcast((1, 128)))
        ada = main.tile([128, 36], F32)
        nc.sync.dma_start(ada, dram["ada_c"].ap())
        n1_sb = main.tile([128, DC], F32)
        nc.sync.dma_start(n1_sb, dram["n1_c"].ap())
        n2_sb = main.tile([128, DC], F32)
        nc.sync.dma_start(n2_sb, dram["n2_c"].ap())
        b1_sb = main.tile([128, HC], F32)
        nc.sync.dma_start(b1_sb, dram["b1_c"].ap())
        b2_sb = main.tile([128, DC], F32)
        nc.sync.dma_start(b2_sb, dram["b2_c"].ap())

        sh_msa, sc_msa, g_msa = ada[:, 0:6], ada[:, 6:12], ada[:, 12:18]
        sh_mlp, sc_mlp, g_mlp = ada[:, 18:24], ada[:, 24:30], ada[:, 30:36]
        a1 = main.tile([128, DC], F32)
        nc.vector.scalar_tensor_tensor(a1, in0=sc_msa, scalar=1.0, in1=n1_sb,
                                       op0=OP.add, op1=OP.mult)
        a2 = main.tile([128, DC], F32)
        nc.vector.scalar_tensor_tensor(a2, in0=sc_mlp, scalar=1.0, in1=n2_sb,
                                       op0=OP.add, op1=OP.mult)
        gb2 = main.tile([128, DC], F32)
        nc.vector.tensor_mul(gb2, g_mlp, b2_sb)

        oT = main.tile([128, DC, NQ], MDT, name="oT")
        x2 = main.tile([128, DC, NQ], F32R, name="x2")
        xskip = main.tile([128, DC, NQ], F32R, name="xskip")
        xm2 = main.tile([128, DC, NQ], MDT, name="xm2")
        wout_all = main.tile([128, DC, D], MDT, name="wout")

        attn_ctx = tc.tile_pool(name="attn", bufs=1)
        attn = attn_ctx.__enter__()
        kt = [attn.tile([128, S], MDT, name=f"kt{p}") for p in range(NP)]
        qT = attn.tile([128, NP, NQ], MDT, name="qT")
        v_tiles = [attn.tile([128, 2, H, DH + 4], FP8, name=f"v{tpp}")
                   for tpp in range(NTP // 2)]
        xm_tiles = []

        wvp_ctx = tc.tile_pool(name="wvp", bufs=1)
        wvp = wvp_ctx.__enter__()
        wv_all = wvp.tile([128, DC, D], MDT, name="wv")
        wqk_ctx = tc.tile_pool(name="wqk", bufs=1)
        wqk = wqk_ctx.__enter__()
        wq_all = wqk.tile([128, DC, D], MDT, name="wq")
        wk_all = wqk.tile([128, DC, D], MDT, name="wk")

        def _ln_stats(psr, x_t, s1_lhs=None, sq_eng=None):
            """LN sums via PE matmuls (1/D folded into the stationary ones
            column); squares on gpsimd (idle engine) unless sq_eng given."""
            eng = sq_eng if sq_eng is not None else nc.vector
            s1 = psr.tile([1, NQ], F32, tag="s1")
            for o in range(DC):
                nc.tensor.matmul(s1, s1_lhs if s1_lhs is not None else sums_col,
                                 x_t[:, o, :],
                                 start=(o == 0), stop=(o == DC - 1))
            s2 = psr.tile([1, NQ], F32, tag="s2")
            for o in range(DC):
                sq = vt.tile([128, NQ], MDT, tag="sq", bufs=2)
                eng.tensor_mul(sq, x_t[:, o, :], x_t[:, o, :])
                nc.tensor.matmul(s2, sums_col, sq,
                                 start=(o == 0), stop=(o == DC - 1))
            return s1, s2

        def _ln_finish(psr, s1, s2, x_t, xm, a_col, sh_col):
            """rstd = exp(-0.5*ln(var+eps)) on ACT (one table set), rows
            broadcast across partitions with K=1 matmuls into PSUM, then
            xm[:, o, :] = ((x - mu) * rstd) * a[d] + sh[d] (DVE + ACT)."""
            mu = rows.tile([1, NQ], F32R, tag="mu")
            nc.vector.tensor_copy(mu, s1)
            musq = rows.tile([1, NQ], F32R, tag="musq")
            nc.vector.tensor_mul(musq, mu, mu)
            var = rows.tile([1, NQ], F32R, tag="var")
            nc.vector.scalar_tensor_tensor(var, in0=s2, scalar=EPS, in1=musq,
                                           op0=OP.add, op1=OP.subtract)
            lnv = rows.tile([1, NQ], F32R, tag="lnv")
            nc.scalar.activation(lnv, var, AF.Ln)
            rstd = rows.tile([1, NQ], MDT, tag="rstd")
            nc.scalar.activation(rstd, lnv, AF.Exp, bias=0.0, scale=-0.5)
            negmr = rows.tile([1, NQ], MDT, tag="negmr")
            nc.vector.scalar_tensor_tensor(negmr, in0=mu, scalar=-1.0,
                                           in1=rstd, op0=OP.mult, op1=OP.mult)
            Rb = psr.tile([128, NQ], F32, tag="Rb")
            nc.tensor.matmul(Rb, bc_row, rstd, start=True, stop=True)
            Mb = psr.tile([128, NQ], F32, tag="Mb")
            nc.tensor.matmul(Mb, bc_row, negmr, start=True, stop=True)
            for o in range(DC):
                t = vt.tile([128, NQ], F32R, tag="tu", bufs=4)
                nc.vector.tensor_mul(t, x_t[:, o, :], Rb)
                u = vt.tile([128, NQ], F32R, tag="tu", bufs=4)
                nc.vector.tensor_add(u, t, Mb)
                nc.scalar.activation(xm[:, o, :], u, AF.Identity,
                                     bias=sh_col[:, o:o + 1],
                                     scale=a_col[:, o:o + 1])

        def _scores_exp(p, kc, sc_pool, et2):
            """Paired score matmuls (even head rows 0-63, odd head rows
            64-127 -> adjacent MMs in different PSUM banks, concurrent in
            the PE array via row tiling) + one [128, 1024] exp into half of
            a kc-pair group tile (fp8: feeds the DoubleRow AV; softmax
            quantization is normalization-protected since the denominator
            sums the same quantized values)."""
            sc = sc_pool.tile([128, 2 * NQ], F32, tag="sc", bufs=2)
            nc.tensor.matmul(sc[:, 0:NQ],
                             kt[p][0:DH, kc * 128:(kc + 1) * 128],
                             qT[0:DH, p, :], start=True, stop=True)
            nc.tensor.matmul(sc[:, NQ:2 * NQ],
                             kt[p][DH:128, kc * 128:(kc + 1) * 128],
                             qT[DH:128, p, :], start=True, stop=True)
            if kc % 2 == 0:
                et2[kc // 2] = attn.tile([128, 2, 2 * NQ], FP8, tag="et",
                                         bufs=3, name=f"et2_{p}_{kc}")
            nc.scalar.activation(et2[kc // 2][:, kc % 2, :], sc,
                                 AF.Exp, bias=0.0, scale=0.125)

        def _k_proj(p, ch, xm):
            ps = psmm.tile([128, NQ], F32, tag="mm")
            for o in range(DC):
                nc.tensor.matmul(ps, wk_all[:, o, p * 128:(p + 1) * 128],
                                 xm[:, o, :], start=(o == 0), stop=(o == DC - 1))
            nc.scalar.copy(kt[p][:, ch * NQ:(ch + 1) * NQ], ps)

        def _q_proj(p, xm0):
            ps = psmm.tile([128, NQ], F32, tag="mm")
            for o in range(DC):
                nc.tensor.matmul(ps, wq_all[:, o, p * 128:(p + 1) * 128],
                                 xm0[:, o, :], start=(o == 0), stop=(o == DC - 1))
            nc.vector.tensor_copy(qT[:, p, :], ps)

        def _v_proj(tp, half):
            """V for key-chunk tp, heads [6*half, 6*half+6), natural layout."""
            ch, sub = tp // 4, tp % 4
            ps = psmm.tile([128, 384], F32, tag="mm")
            for o in range(DC):
                nc.tensor.matmul(
                    ps, xm_tiles[ch][:, o, sub * 128:(sub + 1) * 128],
                    wv_all[:, o, half * 384:(half + 1) * 384],
                    start=(o == 0), stop=(o == DC - 1))
            nc.vector.tensor_copy(
                v_tiles[tp // 2][:, tp % 2, half * 6:(half + 1) * 6, 0:DH],
                ps.rearrange("p (h d) -> p h d", h=6))

        def _av_step(p, e, g, po, et2):
            """One DoubleRow AV matmul for head 2p+e over key-chunk pair g
            (K=256: both kc of the group in one pass)."""
            h = 2 * p + e
            nc.tensor.matmul(po, v_tiles[g][:, :, h, 0:DH + 1],
                             et2[g][:, :, e * NQ:(e + 1) * NQ],
                             start=(g == 0), stop=(g == NTP // 2 - 1),
                             perf_mode=mybir.MatmulPerfMode.DoubleRow)

        def _po_evac(po):
            """Copy AV output + denominator row out of PSUM immediately so
            the po bank frees for the next pair."""
            poS = vt.tile([DH, NQ], MDT, tag="poS", bufs=2)
            nc.vector.tensor_copy(poS, po[0:DH, :])
            denS = vt.tile([1, NQ], F32, tag="denS", bufs=2)
            nc.vector.tensor_copy(denS, po[DH:DH + 1, :])
            return poS, denS

        def _head_tail(p, e, poS, denS):
            """Normalize: oT[head rows, p, :] = poS / den.  1/den on ACT as
            exp(-ln(den)) (same table set as softmax exp) for pairs where
            ACT has slack, DVE reciprocal mid-window; the row is broadcast
            across partitions with a K=1 matmul into PSUM (the DRAM-bounce
            broadcast costs ~7us of SP issue time and serialized the
            attention->out-projection transition)."""
            rrow_bf = vt.tile([1, NQ], MDT, tag="rrowb", bufs=2)
            lnd = vt.tile([1, NQ], F32R, tag="lnd", bufs=1)
            nc.scalar.activation(lnd, denS, AF.Ln)
            nc.scalar.activation(rrow_bf, lnd, AF.Exp, bias=0.0,
                                 scale=-1.0)
            rb_ps = psmm.tile([DH, NQ], F32, tag="mm")
            nc.tensor.matmul(rb_ps, bc_row[:, 0:DH], rrow_bf,
                             start=True, stop=True)
            nc.vector.tensor_mul(oT[e * DH:(e + 1) * DH, p, :], poS, rb_ps)

        # ones column for the softmax denominator
        for tpp in range(NTP // 2):
            nc.vector.tensor_copy(v_tiles[tpp][:, :, :, DH:DH + 1],
                                  ones_f32.to_broadcast((128, 2, H, 1)))

        # =========== phase A: LN1 software-pipelined with K-projection
        # (K/Q for chunk ch-1 run on the PE while chunk ch's modulate runs
        # on DVE/ACT, so the PE never waits for the LN chain)
        q_sched = {0: [], 1: [0, 1], 2: [2, 4], 3: [3, 5]}
        with tc.tile_pool(name="psA", bufs=1, space="PSUM") as psA:
            for ch in range(NCH):
                if ch == 0:
                    x_t = x0
                else:
                    x_t = main.tile([128, DC, NQ], MDT, name=f"x{ch}",
                                    tag="xr", bufs=2)
                    nc.sync.dma_start(x_t, xTb_r[:, :, ch * NQ:(ch + 1) * NQ])
                if ch == 0:
                    nc.sync.dma_start(wk_all[:, :, 0:384],
                                        wqkv_r[:, :, D:D + 384])
                    nc.sync.dma_start(wk_all[:, :, 384:768],
                                        wqkv_r[:, :, D + 384:2 * D])
                elif ch == 1:
                    nc.sync.dma_start(wq_all, wqkv_r[:, :, 0:D])
                elif ch == 2:
                    nc.sync.dma_start(wv_all, wqkv_r[:, :, 2 * D:3 * D])
                    nc.sync.dma_start(wout_all, wout_r)
                xm = attn.tile([128, DC, NQ], MDT, name=f"xm{ch}")
                s1, s2 = _ln_stats(psA, x_t)
                _ln_finish(psA, s1, s2, x_t, xm, a1, sh_msa)
                xm_tiles.append(xm)
                if ch >= 1:
                    for p in range(NP):
                        _k_proj(p, ch - 1, xm_tiles[ch - 1])
                    for p in q_sched[ch]:
                        _q_proj(p, xm_tiles[0])
                    if ch == 3:
                        for tp in range(4):
                            _v_proj(tp, 0)
            for p in range(NP):
                _k_proj(p, NCH - 1, xm_tiles[NCH - 1])
        wqk_ctx.__exit__(None, None, None)

        # =========== pair-major attention: scores+exp dovetail with the
        # same pair's AV (lag 2) and just-in-time V projection (p0: half0,
        # p1: half1)
        with tc.tile_pool(name="scP", bufs=2, space="PSUM") as scP, \
             tc.tile_pool(name="pop", bufs=2, space="PSUM") as pop:
            for p in range(NP):
                if p == 2:
                    nc.sync.dma_start(xskip, xT_r[:, :, 0:NQ])
                et2 = [None] * (NTP // 2)
                po_e = pop.tile([DH + 1, NQ], F32, tag="po")
                po_o = pop.tile([DH + 1, NQ], F32, tag="po")
                for s in range(NTP + 2):
                    if s < NTP:
                        _scores_exp(p, s, scP, et2)
                        if p == 0 and s >= 4:
                            _v_proj(s, 0)
                        elif p == 1 and s % 2 == 0:
                            _v_proj(s, 1)
                        elif p == 2 and s % 2 == 1:
                            _v_proj(s, 1)
                    if s >= 3 and (s - 3) % 2 == 0:
                        g = (s - 3) // 2
                        _av_step(p, 0, g, po_e, et2)
                        _av_step(p, 1, g, po_o, et2)
                poS_e, den_e = _po_evac(po_e)
                poS_o, den_o = _po_evac(po_o)
                _head_tail(p, 0, poS_e, den_e)
                _head_tail(p, 1, poS_o, den_o)
        wvp_ctx.__exit__(None, None, None)
        attn_ctx.__exit__(None, None, None)

        # =========== out-projection + gated residual -> x2 (f32r), with
        # LN2 stats accumulating as each x2 chunk lands
        with tc.tile_pool(name="psB", bufs=1, space="PSUM") as psB:
            s1B = psB.tile([1, NQ], F32, tag="s1")
            s2B = psB.tile([1, NQ], F32, tag="s2")
            for mo in range(DC):
                ps = psmm.tile([128, NQ], F32, tag="mm")
                for o in range(DC):
                    nc.tensor.matmul(ps,
                                     wout_all[:, o, mo * 128:(mo + 1) * 128],
                                     oT[:, o, :],
                                     start=(o == 0), stop=(o == DC - 1))
                nc.vector.scalar_tensor_tensor(
                    x2[:, mo, :], in0=ps, scalar=g_msa[:, mo:mo + 1],
                    in1=xskip[:, mo, :], op0=OP.mult, op1=OP.add)
                nc.tensor.matmul(s1B, sums_colr, x2[:, mo, :],
                                 start=(mo == 0), stop=(mo == DC - 1))
                sq = vt.tile([128, NQ], MDT, tag="sq", bufs=2)
                nc.vector.tensor_mul(sq, x2[:, mo, :], x2[:, mo, :])
                nc.tensor.matmul(s2B, sums_col, sq,
                                 start=(mo == 0), stop=(mo == DC - 1))

            # =========== LN2 + modulation -> xm2
            _ln_finish(psB, s1B, s2B, x2, xm2, a2, sh_mlp)

        # =========== MLP: MLP1+gelu / MLP2 interleaved per hidden chunk
        gfunc = (AF.Gelu_apprx_tanh if gelu_mode == "fused" else AF.Gelu)
        with tc.tile_pool(name="mlpw", bufs=1) as mlpw, \
             tc.tile_pool(name="pso", bufs=1, space="PSUM") as pso:
            w1c = [None] * DC

            def w1dma(wt):
                w1c[wt] = mlpw.tile([128, DC, NQ], MDT, tag="w1", bufs=3,
                                    name=f"w1c{wt}")
                nc.sync.dma_start(w1c[wt],
                                  wm1_r[:, :, wt * NQ:(wt + 1) * NQ])

            w1dma(0)
            w1dma(1)
            w2_all = mlpw.tile([128, HC, D], MDT, name="w2")
            nc.sync.dma_start(w2_all, wm2_r)
            h_tiles = [mlpw.tile([128, NQ], MDT, name=f"h{ho}")
                       for ho in range(HC)]
            for ho in range(HC):
                wt = ho // 4
                if ho % 4 == 0 and wt + 2 < DC:
                    w1dma(wt + 2)
                ps = psmm.tile([128, NQ], F32, tag="mm")
                for o in range(DC):
                    nc.tensor.matmul(
                        ps, w1c[wt][:, o, (ho % 4) * 128:(ho % 4 + 1) * 128],
                        xm2[:, o, :], start=(o == 0), stop=(o == DC - 1))
                nc.scalar.activation(h_tiles[ho], ps, gfunc,
                                     bias=b1_sb[:, ho:ho + 1], scale=1.0)
            # MLP2 wt-major: each output chunk finishes (and its epilogue +
            # store DMA starts) after 1/6 of MLP2 instead of all at the end
            for wt2 in range(DC):
                ps2 = pso.tile([128, NQ], F32, tag="po2", bufs=2)
                for ho in range(HC):
                    nc.tensor.matmul(ps2,
                                     w2_all[:, ho, wt2 * 128:(wt2 + 1) * 128],
                                     h_tiles[ho],
                                     start=(ho == 0), stop=(ho == HC - 1))
                tmp = vt.tile([128, NQ], F32R, tag="tu", bufs=4)
                nc.scalar.activation(tmp, ps2, AF.Identity,
                                     bias=gb2[:, wt2:wt2 + 1],
                                     scale=g_mlp[:, wt2:wt2 + 1])
                nc.vector.tensor_add(x2[:, wt2, :], tmp, x2[:, wt2, :])
                nc.sync.dma_start(outT_r[:, wt2, :], x2[:, wt2, :])


def _fix_module_for_walrus(nc):
    """Workarounds for this container's walrus build:
    (a) it rejects >1 sync-wait per instruction ("Too many sync wait
        commands") -> hoist extra waits onto NoOp carrier instructions;
    (b) it rejects custom Pool InstISA ("ISA wrong length") -> expand the
        tail EVENT_SEMAPHORE_RANGE_CLEAR into per-sem sem-sub-imm updates
        using the final values observed in earlier waits.
    """
    import bass_rust
    nid = [0]

    def carrier(engine, wait):
        nop = mybir.InstNoOp(name=f"wsplit_{nid[0]}", ins=[], outs=[])
        nid[0] += 1
        nop.engine = engine
        nop.sync_info = mybir.SyncInfo(on_wait=[wait], on_update=[])
        return nop

    for f in nc.m.functions:
        new_blocks = []
        for bb in f.blocks:
            sem_final = {}
            out = []
            for inst in bb.instructions:
                si = inst.sync_info
                if si is not None:
                    for w in si.on_wait:
                        if w.sync_type == "semaphore" and w.wait_mode == "sem-ge-imm":
                            sem_final[w.id] = max(sem_final.get(w.id, 0),
                                                  w.wait_value)
                if (type(inst).__name__ == "InstISA"
                        and getattr(inst, "op_name", "") ==
                        "EVENT_SEMAPHORE_RANGE_CLEAR"):
                    ad = inst.ant_dict
                    lo, hi = ad["range_first"], ad["range_last"]
                    waits = list(si.on_wait) if si else []
                    for w in waits:
                        out.append(carrier(inst.engine, w))
                    for sem_id in range(lo, hi + 1):
                        v = sem_final.get(sem_id, 0)
                        if v == 0:
                            continue
                        ev = mybir.InstEventSemaphore(
                            name=f"semclr_{nid[0]}", ins=[], outs=[])
                        nid[0] += 1
                        ev.engine = inst.engine
                        ev.sync_info = mybir.SyncInfo(
                            on_wait=[],
                            on_update=[mybir.SyncUpdate(
                                sync_type="semaphore", id=sem_id,
                                ant_name=f"clr{sem_id}",
                                update_mode="sem-sub-imm", update_value=v,
                                update_reg=None)])
                        out.append(ev)
                    continue
                if type(inst).__name__ == "InstISA":
                    raise RuntimeError(
                        f"unsupported InstISA {getattr(inst, 'op_name', '?')}")
                waits = list(si.on_wait) if si else []
                if len(waits) > 1:
                    for w in waits[:-1]:
                        out.append(carrier(inst.engine, w))
                    inst.sync_info = mybir.SyncInfo(
                        on_wait=waits[-1:], on_update=list(si.on_update))
                out.append(inst)
            nbb = bass_rust.BasicBlock(name=bb.name, instructions=out)
            for attr in ("IsExit", "IsLoopEntry", "IsPredicated"):
                try:
                    setattr(nbb, attr, getattr(bb, attr))
                except Exception:
                    pass
            new_blocks.append(nbb)
        f.blocks = new_blocks
    return nc


def _build_nc(gelu_mode="fused", prec="bf16"):
    nc = bass.Bass(
        "TRN2", target_bir_lowering=False, debug=False, enable_asserts=False,
        num_devices=8,
    )
    WDT = BF16
    shapes = {
        "xT": ([D, S], F32R),
        "xTb": ([D, S], BF16),
        "ada_c": ([128, 36], F32),
        "n1_c": ([128, DC], F32),
        "n2_c": ([128, DC], F32),
        "w_qkvT": ([D, 3 * D], WDT),
        "w_outT": ([D, D], WDT),
        "w_m1T": ([D, HID], WDT),
        "b1_c": ([128, HC], F32),
        "w_m2T": ([HID, D], WDT),
        "b2_c": ([128, DC], F32),
    }
    dram = {k: nc.dram_tensor(k, shp, dt, kind="ExternalInput")
            for k, (shp, dt) in shapes.items()}
    dram["outT"] = nc.dram_tensor("outT", [D, NQ], F32R, kind="ExternalOutput")
    with tile.TileContext(nc) as tc:
        _body(tc, dram, gelu_mode, prec)
    return nc


def _ensure_fixed(nc):
    if not getattr(nc, "_walrus_fixed", False):
        _fix_module_for_walrus(nc)
        nc._walrus_fixed = True
    return nc


_NC_CACHE = {}


def _get_nc(gelu_mode="fused", prec="bf16"):
    key = (gelu_mode, prec)
    if key not in _NC_CACHE:
        _NC_CACHE[key] = _build_nc(gelu_mode, prec)
    return _NC_CACHE[key]


def _colpack(v, nch):
    """[nch*128] vector -> [128, nch] column-packed (col jo = v[jo*128+p])."""
    return np.ascontiguousarray(np.asarray(v, np.float32).reshape(nch, 128).T)


def make_in_maps(inputs, prec="bf16"):
    import ml_dtypes
    wdt = ml_dtypes.bfloat16
    x = np.asarray(inputs["x"], np.float32)
    c = np.asarray(inputs["c"], np.float32)
    w_ada = np.asarray(inputs["w_ada"], np.float32)
    b_ada = np.asarray(inputs["b_ada"], np.float32)
    # AdaLN modulation vectors: tiny (2x 4608x768) matmul, replicated per the
    # sharding hint; column-packed per batch.
    ada = c @ w_ada.T + b_ada                      # (2, 4608)
    tr = lambda w: np.ascontiguousarray(np.asarray(w, np.float32).T.astype(wdt))
    base = {
        "n1_c": _colpack(inputs["norm1_w"], DC),
        "n2_c": _colpack(inputs["norm2_w"], DC),
        "w_qkvT": tr(inputs["w_qkv"]),
        "w_outT": tr(inputs["w_out"]),
        "w_m1T": tr(inputs["w_mlp1"]),
        "b1_c": _colpack(inputs["b_mlp1"], HC),
        "w_m2T": tr(inputs["w_mlp2"]),
        "b2_c": _colpack(inputs["b_mlp2"], DC),
    }
    in_maps = []
    for core in range(8):
        b, k = core // 4, core % 4
        xb = np.roll(x[b], -NQ * k, axis=0)        # my queries first
        m = dict(base)
        m["xT"] = np.ascontiguousarray(xb.T)
        m["xTb"] = np.ascontiguousarray(xb.T.astype(wdt))
        m["ada_c"] = _colpack(ada[b], 36)
        in_maps.append(m)
    return in_maps


def assemble_output(results):
    out = np.empty((2, S, D), np.float32)
    for core in range(8):
        b, k = core // 4, core % 4
        out[b, NQ * k:NQ * (k + 1)] = results[core]["outT"].T
    return out


def kernel(**inputs):
    prec = "bf16"
    nc = _ensure_fixed(_get_nc(prec=prec))
    in_maps = make_in_maps(inputs, prec=prec)
    res = run_bass_kernel_spmd(nc, in_maps, core_ids=list(range(8)))
    return assemble_output(res.results)


if __name__ == "__main__":
    _get_nc()
    print("build ok")
